# revision 9
# baseline (speedup 1.0000x reference)
# Trainium2 Bass kernel for DMOR (dynamic mixture-of-operators routing).
#
# Reference computation (per image):
#   op_feats = [x, conv3x3(x), conv3x3_dilated2(x), avgpool3x3(x), dwconv3x3(x)]
#   z = spatial_router(x) + global_router(GAP(x))          # [5, H, W]
#   w = softmax(z, axis=0); top-2 mask + renormalize (eps=1e-6)
#   out = sum_n w_n * op_feats_n
#
# Sharding: data-parallel over batch B=8 across 8 NeuronCores (1 image/core),
# weights replicated. One SPMD program; per-core in_maps differ only in x.
#
# Device structure (per core, x:[64,128,128] -> out:[64,128,128]):
#   - Stencil ops are 9-shift accumulating PSUM matmuls in bf16 (~1e-3 rel
#     err; convs only — the router stays exact). Shift-pairs pack K=128 via a
#     host-built dual copy: partitions 0-63 = padded x, 64-127 = same shifted
#     left 2 cols. The padded image ships as 4 row-bands (38 rows each, 5-row
#     halo overlap) so conv matmuls start after the first band lands.
#     3 PSUM groups per 512-px tile:
#       bankA[128,512] = [conv3x3 | avgpool]   (avg = (1/9)*I diagonal)
#       bankB[128,512] = [dwconv | identity]   (identity = I at center shift)
#       bankC[64,512]  = dilated conv
#   - Router z: exact fp32 matmuls from a separate fp32 x load (top-2
#     selection is discontinuous; z gaps go down to ~1e-7). z is drained
#     unbiased (no GAP dependency); the data-dependent global bias is applied
#     in pixel-major form via a tiny broadcast matmul before the softmax.
#   - z moves to pixel-major [128,5,128] via a DRAM bounce (partition-crossing
#     SBUF->SBUF DMAs are miscompiled, measured); softmax/top-2 is ~14
#     whole-image DVE ops; w bounces back flat.
#   - w is replicated across channel partitions by 0-stride DRAM->SBUF DMA
#     reads (no PE broadcast). Mix is 3 fused (f + bias) * w DVE ops; the two
#     128-partition products pre-add on DVE; 2 identity matmuls accumulate the
#     5-term sum in PSUM. po(t) is emitted after banks(t+1) so the PE never
#     stalls on the DVE mixes.
import numpy as np

B, C, H, W = 8, 64, 128, 128
HW = H * W
N_OPS = 5
HID = 16
PAD = 2
HP, WP = H + 2 * PAD, W + 2 * PAD  # 132, 132
TPX = 512                          # pixels per tile (4 image rows)
NT = HW // TPX                     # 32 tiles
ROWS_PER_TILE = TPX // W           # 4
NBAND = 4
BROWS = 38                         # padded rows per band (32 + 6 halo)
BSTRIDE = 32
TPB = NT // NBAND                  # tiles per band (8)
EPS = 1e-6
NEG_BIG = -1.0e30

_CACHE = {}


def _bf16():
    import ml_dtypes
    return ml_dtypes.bfloat16


def _host_consts(w3, b3, wd, bd, wdw, bdw, gr_w1, gr_w2, gr_b2, sr_w, sr_b):
    """Precompute all stationary matmul operands / bias vectors in numpy.

    Matmul i in 0..5 per op group: i<3 are K=128 shift-pairs (kh=i, partition
    half 0 = col-shift kw0, half 1 = kw1 via the +2-shifted x copy); i>=3 are
    K=64 unpaired (kh=i-3, remaining kw)."""
    f32 = np.float32
    bf16 = _bf16()
    w3 = np.asarray(w3, f32); wd = np.asarray(wd, f32); wdw = np.asarray(wdw, f32)
    eye = np.eye(C, dtype=f32)
    avg = eye * f32(1.0 / 9.0)

    A = np.zeros((128, 6, 128), f32)
    Bd = np.zeros((128, 6, 128), f32)
    Cd = np.zeros((128, 6, 64), f32)
    for kh in range(3):
        # paired: conv3/dw cols (kw=0 | kw=2); dilated cols (kw=0 | kw=1)
        A[0:64, kh, 0:64] = w3[:, :, kh, 0].T
        A[64:128, kh, 0:64] = w3[:, :, kh, 2].T
        A[0:64, kh, 64:128] = avg
        A[64:128, kh, 64:128] = avg
        Bd[0:64, kh, 0:64] = eye * wdw[:, 0, kh, 0][None, :]
        Bd[64:128, kh, 0:64] = eye * wdw[:, 0, kh, 2][None, :]
        Cd[0:64, kh, :] = wd[:, :, kh, 0].T
        Cd[64:128, kh, :] = wd[:, :, kh, 1].T
        # unpaired: conv3/dw kw=1; dilated kw=2
        A[0:64, 3 + kh, 0:64] = w3[:, :, kh, 1].T
        A[0:64, 3 + kh, 64:128] = avg
        Bd[0:64, 3 + kh, 0:64] = eye * wdw[:, 0, kh, 1][None, :]
        Cd[0:64, 3 + kh, :] = wd[:, :, kh, 2].T
    Bd[0:64, 4, 64:128] = eye  # identity op = center shift (kh=1, kw=1)

    R = np.ascontiguousarray(np.asarray(sr_w, f32)[:, :, 0, 0].T)          # [C, 5]
    G1 = np.ascontiguousarray((np.asarray(gr_w1, f32)[:, :, 0, 0] / HW).T)  # [C, HID]
    G2 = np.ascontiguousarray(np.asarray(gr_w2, f32)[:, :, 0, 0].T)        # [HID, 5]
    HBIAS = (np.asarray(sr_b, f32) + np.asarray(gr_b2, f32)).reshape(N_OPS, 1)

    SUM128 = np.ascontiguousarray(np.concatenate([eye, eye], axis=0))  # [128, 64]

    BIAS13 = np.concatenate([np.asarray(b3, f32), np.zeros(C, f32)]).reshape(128, 1)
    BIAS40 = np.concatenate([np.asarray(bdw, f32), np.zeros(C, f32)]).reshape(128, 1)
    BIAS2 = np.asarray(bd, f32).reshape(C, 1)

    return {
        "cA": np.ascontiguousarray(A.reshape(128, 6 * 128).astype(bf16)),
        "cB": np.ascontiguousarray(Bd.reshape(128, 6 * 128).astype(bf16)),
        "cC": np.ascontiguousarray(Cd.reshape(128, 6 * 64).astype(bf16)),
        "cR": R, "cG1": G1, "cG2": G2, "cHB": HBIAS,
        "cSUM128": SUM128, "cSUM64": eye,
        "cBIAS13": BIAS13, "cBIAS40": BIAS40, "cBIAS2": BIAS2,
        "cONES": np.ones((N_OPS, 128), f32),
        "cI5": np.ascontiguousarray(np.eye(N_OPS, dtype=f32)),
    }


def _host_pad(x_img):
    """Banded dual-shift padded image [128, NBAND, BROWS*WP] bf16.

    Partitions 0-63 = zero-padded x; 64-127 = same shifted left 2 cols.
    Band b holds padded rows [32b, 32b+38) (zero past the image)."""
    bf16 = _bf16()
    xp = np.zeros((128, HP + 4, WP), np.float32)
    xp[0:C, PAD:PAD + H, PAD:PAD + W] = x_img
    xp[C:128, PAD:PAD + H, 0:W] = x_img
    xb = np.zeros((128, NBAND, BROWS, WP), np.float32)
    for b in range(NBAND):
        xb[:, b] = xp[:, BSTRIDE * b:BSTRIDE * b + BROWS]
    return np.ascontiguousarray(
        xb.reshape(128, NBAND, BROWS * WP).astype(bf16))


def _build_program():
    import concourse.bass as bass
    import concourse.bacc as bacc
    import concourse.tile as tile
    import concourse.mybir as mybir
    from contextlib import ExitStack

    dt = mybir.dt
    f32 = dt.float32
    f32r = dt.float32r
    bf16 = dt.bfloat16
    AF = mybir.ActivationFunctionType
    ALU = mybir.AluOpType
    AX = mybir.AxisListType

    nc = bacc.Bacc("TRN2", target_bir_lowering=False, debug=False)

    xin = nc.dram_tensor("xin", [C, HW], f32, kind="ExternalInput")
    xbd = nc.dram_tensor("xbd", [128, NBAND * BROWS * WP], bf16,
                         kind="ExternalInput")
    dr = {}
    for name, shape, d in [
        ("cA", [128, 6 * 128], bf16), ("cB", [128, 6 * 128], bf16),
        ("cC", [128, 6 * 64], bf16),
        ("cR", [C, N_OPS], f32), ("cG1", [C, HID], f32),
        ("cG2", [HID, N_OPS], f32), ("cHB", [N_OPS, 1], f32),
        ("cSUM128", [128, 64], f32), ("cSUM64", [64, 64], f32),
        ("cBIAS13", [128, 1], f32), ("cBIAS40", [128, 1], f32),
        ("cBIAS2", [64, 1], f32),
        ("cONES", [N_OPS, 128], f32), ("cI5", [N_OPS, N_OPS], f32),
    ]:
        dr[name] = nc.dram_tensor(name, shape, d, kind="ExternalInput")
    yout = nc.dram_tensor("yout", [C, HW], f32, kind="ExternalOutput")

    def b5(t):
        # broadcast a [128,128] tile across the 5-map free dim -> [128,5,128]
        return bass.AP(tensor=t.tensor, offset=t.offset,
                       ap=[list(t.ap[0]), [0, N_OPS], list(t.ap[1])])

    def rr(ap):
        return ap.bitcast(f32r)

    with tile.TileContext(nc) as tc, ExitStack() as ctx:
        consts = ctx.enter_context(tc.tile_pool(name="consts", bufs=1))
        xpool = ctx.enter_context(tc.tile_pool(name="xp", bufs=1))
        zpool = ctx.enter_context(tc.tile_pool(name="z", bufs=1))
        gaps = ctx.enter_context(tc.tile_pool(name="gaps", bufs=1))
        zchunk = ctx.enter_context(tc.tile_pool(name="zchunk", bufs=2))
        drpool = ctx.enter_context(tc.tile_pool(name="drbounce", bufs=1,
                                                space="DRAM"))

        # ---- constant tiles -------------------------------------------------
        wA = consts.tile([128, 6, 128], bf16)
        wB = consts.tile([128, 6, 128], bf16)
        wC = consts.tile([128, 6, 64], bf16)
        s128 = consts.tile([128, 64], f32r)
        s64 = consts.tile([C, 64], f32r)
        wR = consts.tile([C, N_OPS], f32)
        wG1 = consts.tile([C, HID], f32)
        wG2 = consts.tile([HID, N_OPS], f32)
        hbv = consts.tile([N_OPS, 1], f32)
        bias13 = consts.tile([128, 1], f32)
        bias40 = consts.tile([128, 1], f32)
        bias2 = consts.tile([C, 1], f32)
        ones5 = consts.tile([N_OPS, 128], f32)
        i5 = consts.tile([N_OPS, N_OPS], f32)

        nc.sync.dma_start(out=wA,
                          in_=dr["cA"][:, :].rearrange("c (s m) -> c s m", s=6))
        nc.sync.dma_start(out=wB,
                          in_=dr["cB"][:, :].rearrange("c (s m) -> c s m", s=6))
        nc.sync.dma_start(out=wC,
                          in_=dr["cC"][:, :].rearrange("c (s m) -> c s m", s=6))
        nc.sync.dma_start(out=s128, in_=rr(dr["cSUM128"][:, :]))
        nc.sync.dma_start(out=s64, in_=rr(dr["cSUM64"][:, :]))
        for t, name in [(wR, "cR"), (wG1, "cG1"), (wG2, "cG2"), (hbv, "cHB"),
                        (bias13, "cBIAS13"), (bias40, "cBIAS40"),
                        (bias2, "cBIAS2"), (ones5, "cONES"), (i5, "cI5")]:
            nc.sync.dma_start(out=t, in_=dr[name][:, :])

        zpm = zpool.tile([128, N_OPS, 128], f32)
        zb = zpool.tile([128, N_OPS, 128], f32)
        gscr = gaps.tile([C, 2048], f32)
        gparts = gaps.tile([C, 8], f32)
        xg = gaps.tile([C, 1], f32)
        hrelu = gaps.tile([HID, 1], f32)
        biasv = gaps.tile([N_OPS, 1], f32)
        bsb = gaps.tile([N_OPS, 128], f32)
        bias_pm = gaps.tile([128, N_OPS], f32)
        zscr = drpool.tile([N_OPS, HW], f32)
        wscr = drpool.tile([N_OPS, HW], f32)

        m1 = zpool.tile([128, 128], f32)
        m2 = zpool.tile([128, 128], f32)
        zsum = zpool.tile([128, 128], f32)
        e2 = zpool.tile([128, 128], f32)
        eqx = zpool.tile([128, N_OPS, 128], f32)
        em = zpool.tile([128, N_OPS, 128], f32)
        wpm = zpool.tile([128, N_OPS, 128], f32)

        def overn(t):
            return t[:, :, :].rearrange("p n b -> p b n")

        ZC = 1024                  # z drain chunk: 2 tiles, 2 PSUM banks
        XPS = HW // 4              # fp32 x chunk (one tile each)

        # ================= prologue (fp32 x + router) ========================
        with tc.tile_pool(name="stage", bufs=1) as stage, \
             tc.tile_pool(name="ps_r", bufs=2, space="PSUM") as ps_r, \
             tc.tile_pool(name="ps_mlp", bufs=1, space="PSUM") as ps_mlp:
            # exact fp32 x (router + GAP); 4 independent tiles so the router
            # starts after the first lands
            xps = [stage.tile([C, XPS], f32, tag=f"xps{k}", name=f"xps{k}")
                   for k in range(4)]
            for k in range(4):
                nc.sync.dma_start(out=xps[k],
                                  in_=xin[:, k * XPS:(k + 1) * XPS])
            # banded bf16 dual-shift x for the conv path
            xb = [xpool.tile([128, BROWS, WP], bf16, tag=f"xb{b}",
                             name=f"xb{b}") for b in range(NBAND)]
            xsrc = xbd[:, :].rearrange("p (b r) -> p b r", b=NBAND)
            for b in range(NBAND):
                nc.sync.dma_start(
                    out=xb[b],
                    in_=xsrc[:, b, :].rearrange("p (h w) -> p h w", h=BROWS))

            # spatial router stream (exact fp32, unbiased drain on DVE);
            # bounce to DRAM for the pixel-major relayout
            for ch in range(HW // ZC):
                zps = ps_r.tile([N_OPS, ZC], f32)
                for j in range(ZC // TPX):
                    px0 = ch * ZC + j * TPX
                    nc.tensor.matmul(zps[:, j * TPX:(j + 1) * TPX], wR,
                                     xps[px0 // XPS][:, px0 % XPS:px0 % XPS + TPX],
                                     start=True, stop=True)
                zfl = zchunk.tile([N_OPS, ZC], f32)
                nc.vector.tensor_copy(zfl, zps)
                nc.sync.dma_start(out=zscr[:, ch * ZC:(ch + 1) * ZC], in_=zfl)

            # GAP on ACT (runs parallel to z matmuls), then the tiny MLP on PE
            # (emitted after the z chunks so it doesn't block them)
            for k in range(8):
                nc.scalar.activation(
                    out=gscr, in_=xps[k // 2][:, (k % 2) * 2048:(k % 2 + 1) * 2048],
                    func=AF.Copy, accum_out=gparts[:, k:k + 1])
            nc.vector.tensor_reduce(out=xg, in_=gparts, axis=AX.X, op=ALU.add)
            mlp1 = ps_mlp.tile([HID, 1], f32, tag="mlp")
            nc.tensor.matmul(mlp1, wG1, xg, start=True, stop=True)
            nc.scalar.activation(out=hrelu, in_=mlp1, func=AF.Relu)
            mlp2 = ps_mlp.tile([N_OPS, 1], f32, tag="mlp")
            nc.tensor.matmul(mlp2, wG2, hrelu, start=True, stop=True)
            nc.vector.tensor_add(biasv, mlp2, hbv)
            # replicate biasv across the 128 pixel-major partitions:
            # bsb[n, m] = biasv[n]; bias_pm[m, n] = (bsb^T I5)[m, n]
            nc.vector.tensor_tensor(
                out=bsb, in0=ones5,
                in1=bass.AP(tensor=biasv.tensor, offset=biasv.offset,
                            ap=[list(biasv.ap[0]), [0, 128]]),
                op=ALU.mult)
            pmb = ps_mlp.tile([128, N_OPS], f32, tag="pmb")
            nc.tensor.matmul(pmb, bsb, i5, start=True, stop=True)
            nc.vector.tensor_copy(bias_pm, pmb)

            # load z back pixel-major: zpm[p, n, b] = z[n, 128p + b]
            nc.sync.dma_start(
                out=zpm,
                in_=bass.AP(tensor=zscr.tensor, offset=zscr.offset,
                            ap=[[128, 128], [HW, N_OPS], [1, 128]]))

        # ---- softmax + top-2 (pixel-major, whole image) ---------------------
        nc.vector.tensor_tensor(
            out=zb, in0=zpm,
            in1=bass.AP(tensor=bias_pm.tensor, offset=bias_pm.offset,
                        ap=[list(bias_pm.ap[0]), list(bias_pm.ap[1]), [0, 128]]),
            op=ALU.add)
        nc.vector.tensor_reduce(out=m1, in_=overn(zb), axis=AX.X, op=ALU.max)
        nc.vector.tensor_tensor(out=eqx, in0=zb, in1=b5(m1), op=ALU.is_equal)
        nc.vector.scalar_tensor_tensor(out=eqx, in0=eqx, scalar=NEG_BIG,
                                       in1=zb, op0=ALU.mult, op1=ALU.add)
        nc.vector.tensor_reduce(out=m2, in_=overn(eqx), axis=AX.X, op=ALU.max)
        nc.vector.scalar_tensor_tensor(out=em, in0=b5(m1), scalar=-1.0,
                                       in1=zb, op0=ALU.mult, op1=ALU.add)
        nc.scalar.activation(out=em, in_=em, func=AF.Exp)
        nc.vector.tensor_reduce(out=zsum, in_=overn(em), axis=AX.X, op=ALU.add)
        nc.vector.tensor_tensor(out=eqx, in0=zb, in1=b5(m2), op=ALU.is_ge)
        nc.vector.tensor_tensor(out=em, in0=em, in1=eqx, op=ALU.mult)
        nc.vector.tensor_reduce(out=e2, in_=overn(em), axis=AX.X, op=ALU.add)
        nc.vector.scalar_tensor_tensor(out=e2, in0=zsum, scalar=EPS,
                                       in1=e2, op0=ALU.mult, op1=ALU.add)
        nc.vector.reciprocal(out=e2, in_=e2)
        nc.vector.tensor_tensor(out=wpm, in0=em, in1=b5(e2), op=ALU.mult)
        # bounce w to DRAM flat layout
        nc.sync.dma_start(
            out=bass.AP(tensor=wscr.tensor, offset=wscr.offset,
                        ap=[[128, 128], [HW, N_OPS], [1, 128]]),
            in_=wpm)

        # ---- main loop ------------------------------------------------------
        with tc.tile_pool(name="wrep", bufs=3) as wrep, \
             tc.tile_pool(name="gbuf", bufs=2) as gbuf, \
             tc.tile_pool(name="outst", bufs=2) as outst, \
             tc.tile_pool(name="ps_a", bufs=2, space="PSUM") as ps_a, \
             tc.tile_pool(name="ps_b", bufs=2, space="PSUM") as ps_b, \
             tc.tile_pool(name="ps_c", bufs=2, space="PSUM") as ps_c, \
             tc.tile_pool(name="ps_o", bufs=2, space="PSUM") as ps_o:

            def wdma(t):
                # replicate w maps across channel partitions via 0-stride
                # DRAM reads: w13 = [w1 | w3], w40 = [w4 | w0], w2
                w13 = wrep.tile([128, TPX], f32, tag="w13", name="w13")
                w40 = wrep.tile([128, TPX], f32, tag="w40", name="w40")
                w2 = wrep.tile([64, TPX], f32, tag="w2", name="w2")
                for dst, n_lo, n_hi in ((w13, 1, 3), (w40, 4, 0)):
                    for half, n in ((0, n_lo), (1, n_hi)):
                        nc.sync.dma_start(
                            out=dst[64 * half:64 * (half + 1), :],
                            in_=bass.AP(tensor=wscr.tensor,
                                        offset=wscr.offset + n * HW + t * TPX,
                                        ap=[[0, 64], [1, TPX]]))
                nc.sync.dma_start(
                    out=w2,
                    in_=bass.AP(tensor=wscr.tensor,
                                offset=wscr.offset + 2 * HW + t * TPX,
                                ap=[[0, 64], [1, TPX]]))
                return w13, w40, w2

            def banks(t):
                band, j = t // TPB, t % TPB
                h0 = j * ROWS_PER_TILE
                xt = xb[band]

                def rhsAB(i):
                    kh = i if i < 3 else i - 3
                    p1 = 128 if i < 3 else 64
                    co = 1 if i < 3 else 2
                    return xt[0:p1, 1 + kh + h0:1 + kh + h0 + ROWS_PER_TILE,
                              co:co + W]

                def rhsC(i):
                    kh = i if i < 3 else i - 3
                    p1 = 128 if i < 3 else 64
                    co = 0 if i < 3 else 4
                    return xt[0:p1, 2 * kh + h0:2 * kh + h0 + ROWS_PER_TILE,
                              co:co + W]

                bankA = ps_a.tile([128, TPX], f32)
                bankB = ps_b.tile([128, TPX], f32)
                bankC = ps_c.tile([C, TPX], f32)
                for i in range(6):
                    kk = 128 if i < 3 else 64
                    nc.tensor.matmul(bankA, wA[0:kk, i, :], rhsAB(i),
                                     start=(i == 0), stop=(i == 5))
                for i in range(6):
                    kk = 128 if i < 3 else 64
                    nc.tensor.matmul(bankB, wB[0:kk, i, :], rhsAB(i),
                                     start=(i == 0), stop=(i == 5))
                for i in range(6):
                    kk = 128 if i < 3 else 64
                    nc.tensor.matmul(bankC, wC[0:kk, i, :], rhsC(i),
                                     start=(i == 0), stop=(i == 5))
                return bankA, bankB, bankC

            def mix_and_sum(state):
                t, (bankA, bankB, bankC), (w13, w40, w2) = state
                # mix: g = (f + bias) * w    (outputs rounded to fp32r)
                gA = gbuf.tile([128, TPX], f32r, tag="gA", name="gA")
                gC = gbuf.tile([C, TPX], f32r, tag="gC", name="gC")
                nc.vector.scalar_tensor_tensor(out=gA, in0=bankA, scalar=bias13,
                                               in1=w13, op0=ALU.add,
                                               op1=ALU.mult)
                gB = gbuf.tile([128, TPX], f32r, tag="gB", name="gB")
                nc.vector.scalar_tensor_tensor(out=gB, in0=bankB,
                                               scalar=bias40, in1=w40,
                                               op0=ALU.add, op1=ALU.mult)
                nc.vector.scalar_tensor_tensor(out=gC, in0=bankC,
                                               scalar=bias2, in1=w2,
                                               op0=ALU.add, op1=ALU.mult)
                nc.vector.tensor_add(gA, gA, gB)
                # sum the 5 terms in PSUM via identity matmuls
                po = ps_o.tile([64, TPX], f32, tag="po", name="po")
                nc.tensor.matmul(po, s128, gA, start=True, stop=False)
                nc.tensor.matmul(po, s64, gC, start=False, stop=True)
                och = t // 4
                ost = osts[och]
                nc.scalar.activation(
                    out=ost[:, (t % 4) * TPX:(t % 4 + 1) * TPX], in_=po,
                    func=AF.Copy)
                if t % 4 == 3:
                    nc.sync.dma_start(
                        out=yout[:, och * 4 * TPX:(och + 1) * 4 * TPX], in_=ost)

            osts = {}
            pending = None
            wnext = wdma(0)
            for t in range(NT):
                if t % 4 == 0:
                    osts[t // 4] = outst.tile([C, 4 * TPX], f32, name="ost")
                wcur = wnext
                bk = banks(t)
                if t + 1 < NT:
                    wnext = wdma(t + 1)
                if pending is not None:
                    mix_and_sum(pending)
                pending = (t, bk, wcur)
            mix_and_sum(pending)

    nc.compile()
    return nc


def _get_program():
    if "nc" not in _CACHE:
        _CACHE["nc"] = _build_program()
    return _CACHE["nc"]


def _run(inputs, **spmd_kwargs):
    x = np.ascontiguousarray(np.asarray(inputs["x"], np.float32))
    consts = _host_consts(**{k: inputs[k] for k in
                             ["w3", "b3", "wd", "bd", "wdw", "bdw",
                              "gr_w1", "gr_w2", "gr_b2", "sr_w", "sr_b"]})
    nc = _get_program()

    from concourse.bass_utils import run_bass_kernel_spmd
    in_maps = []
    for i in range(B):
        m = dict(consts)
        m["xin"] = np.ascontiguousarray(x[i].reshape(C, HW))
        m["xbd"] = _host_pad(x[i]).reshape(128, NBAND * BROWS * WP)
        in_maps.append(m)
    res = run_bass_kernel_spmd(nc, in_maps, core_ids=list(range(B)), **spmd_kwargs)
    out = np.stack([res.results[i]["yout"].reshape(C, H, W) for i in range(B)])
    return out.astype(np.float32), res


def kernel(**inputs):
    out, _ = _run(inputs)
    return out
